# revision 1
# baseline (speedup 1.0000x reference)
"""Trainium2 Bass kernel for nn_ContextLayer (gnn_message_passing).

Math (reference):
  g0 = x @ W0.T + b0            [B,S,D]
  g1 = x @ W1.T + b1            [B,S,D]
  out[b,q,e] = tanh( (1/L_b) * sum_k m[b,q] m[b,k] x[b,k,e] sigmoid(g0[b,q,e]+g1[b,k,e]) )

Sharding: 8 cores = 4 batches x 2 e-halves (200 e's each). Each core:
  - computes g0t/g1t = [e, s] gate matrices via PE matmuls (contraction over
    d as partitions, 4 K-chunks of 401 rows: 400 features + 1 mask-penalty
    row that adds BIGNEG*(1-m[s]) so masked s give sigmoid()==0),
  - inner loop over (e-chunk, q): ACT computes sigmoid(g1t + g0t[:,q]) with
    the per-partition bias port (fused add), DVE tensor_tensor_reduce fuses
    the multiply by x[k,e] and the k-reduction into acc[:, q],
  - final tanh(acc * 1/L) on ACT with the per-partition scale port.

Host side only slices/transposes inputs and assembles the output.
"""

import numpy as np
from contextlib import ExitStack

from concourse import bacc, mybir, tile
import concourse.bass as bass
from concourse.bass_utils import run_bass_kernel_spmd

B, S, D = 4, 256, 400
EH = 200                      # e-columns per core
CHUNKS = [(0, 128), (128, 72)]  # (e-local offset, partitions)
KCH = [(0, 128), (128, 128), (256, 128), (384, 17)]  # K-chunks over 401
BIGNEG = np.float32(-1e30)
F32 = mybir.dt.float32
N_CORES = 8

_prog_cache = {}


def _build_program(repeat=1, qb=8, dve_mul=(0, 2, 5, 8, 10, 13), bufs=3, modp=16):
    nc = bacc.Bacc("TRN2", target_bir_lowering=False, debug=False)

    xin = nc.dram_tensor("xin", [401, 256], F32, kind="ExternalInput").ap()
    xtin = nc.dram_tensor("xtin", [200, 256], F32, kind="ExternalInput").ap()
    w0t = nc.dram_tensor("w0t", [401, 200], F32, kind="ExternalInput").ap()
    w1t = nc.dram_tensor("w1t", [401, 200], F32, kind="ExternalInput").ap()
    bias01 = nc.dram_tensor("bias01", [128, 4], F32, kind="ExternalInput").ap()
    invl = nc.dram_tensor("invl", [128, 1], F32, kind="ExternalInput").ap()
    out = nc.dram_tensor("out", [200, 256], F32, kind="ExternalOutput").ap()

    AF = mybir.ActivationFunctionType
    OP = mybir.AluOpType

    with ExitStack() as ctx:
        tc = ctx.enter_context(tile.TileContext(nc))
        if repeat > 1:
            ctx.enter_context(tc.For_i(0, repeat, 1))
        const = ctx.enter_context(tc.tile_pool(name="const", bufs=1))
        psum = ctx.enter_context(tc.tile_pool(name="psum", bufs=1, space="PSUM"))
        tpool = ctx.enter_context(tc.tile_pool(name="t", bufs=bufs))

        # ---- loads ----
        rhs = []
        for k0, kn in KCH:
            t = const.tile([kn, 256], F32, tag=f"rhs{k0}")
            nc.sync.dma_start(t[:], xin[k0 : k0 + kn, :])
            rhs.append(t)
        wts = []
        for gi, wsrc in enumerate([w0t, w1t]):
            chunks = []
            for k0, kn in KCH:
                t = const.tile([kn, 200], F32, tag=f"w{gi}_{k0}")
                nc.sync.dma_start(t[:], wsrc[k0 : k0 + kn, :])
                chunks.append(t)
            wts.append(chunks)
        biases = const.tile([128, 4], F32, tag="biases")
        nc.sync.dma_start(biases[:], bias01[:])
        invlt = const.tile([128, 1], F32, tag="invlt")
        nc.sync.dma_start(invlt[:], invl[:])
        xt = []
        for ci, (e0, pn) in enumerate(CHUNKS):
            t = const.tile([pn, 256], F32, tag=f"xt{ci}")
            nc.sync.dma_start(t[:], xtin[e0 : e0 + pn, :])
            xt.append(t)

        # ---- gates: g{0,1}t[e_chunk, s] = W.T @ x.T (+bias, +mask penalty) ----
        gt = [[None, None], [None, None]]  # [gi][ci]
        for ci, (e0, pn) in enumerate(CHUNKS):
            for gi in range(2):
                ps = psum.tile([pn, 256], F32, tag=f"ps{gi}{ci}")
                for kci, (k0, kn) in enumerate(KCH):
                    nc.tensor.matmul(
                        ps[:],
                        wts[gi][kci][:, e0 : e0 + pn],
                        rhs[kci][:],
                        start=(kci == 0),
                        stop=(kci == len(KCH) - 1),
                    )
                gs = const.tile([pn, 256], F32, tag=f"g{gi}t{ci}")
                nc.scalar.activation(
                    gs[:], ps[:], AF.Identity,
                    bias=biases[0:pn, 2 * gi + ci : 2 * gi + ci + 1],
                )
                gt[gi][ci] = gs

        # ---- main loop: acc[e, q] = sum_k sigmoid(g1t[e,k] + g0t[e,q]) * x[k,e] ----
        # Per q-block of QB: QB biased sigmoids (ACT) into a wide tile, one
        # big multiply vs broadcast x (DVE or Pool), one segmented reduce (DVE).
        QB = qb
        NBLK = 256 // QB
        # DVE takes dve_mul of every modp blocks' multiplies, Pool the rest
        DVE_MUL = set(dve_mul)
        accs = []
        for ci, (e0, pn) in enumerate(CHUNKS):
            acc = const.tile([pn, 256], F32, tag=f"acc{ci}")
            xt_b = (
                xt[ci][:]
                .rearrange("p (o k) -> p o k", o=1)
                .broadcast_to((pn, QB, 256))
            )
            for bi in range(NBLK):
                tw = tpool.tile([pn, QB * 256], F32, tag=f"tw{ci}")
                for j in range(QB):
                    q = bi * QB + j
                    nc.scalar.activation(
                        tw[:, j * 256 : (j + 1) * 256], gt[1][ci][:],
                        AF.Sigmoid, bias=gt[0][ci][:, q : q + 1],
                    )
                prod = tpool.tile([pn, QB * 256], F32, tag=f"prod{ci}")
                tw3 = tw[:].rearrange("p (q k) -> p q k", q=QB)
                prod3 = prod[:].rearrange("p (q k) -> p q k", q=QB)
                if (bi % modp) in DVE_MUL:
                    nc.vector.tensor_tensor(out=prod3, in0=tw3, in1=xt_b, op=OP.mult)
                else:
                    nc.gpsimd.tensor_tensor(out=prod3, in0=tw3, in1=xt_b, op=OP.mult)
                nc.vector.tensor_reduce(
                    out=acc[:, bi * QB : (bi + 1) * QB],
                    in_=prod3,
                    axis=mybir.AxisListType.X,
                    op=OP.add,
                )
            accs.append(acc)

        # ---- finalize: out = tanh(acc / L) ----
        for ci, (e0, pn) in enumerate(CHUNKS):
            res = const.tile([pn, 256], F32, tag=f"res{ci}")
            nc.scalar.activation(
                res[:], accs[ci][:], AF.Tanh, scale=invlt[0:pn, :]
            )
            nc.sync.dma_start(out[e0 : e0 + pn, :], res[:])

    nc.compile()
    return nc


def _get_program():
    if "nc" not in _prog_cache:
        _prog_cache["nc"] = _build_program()
    return _prog_cache["nc"]


def _make_in_maps(x, m, W0, b0, W1, b1):
    maskrow = (1.0 - m).astype(np.float32)  # [B, S]
    L = m.sum(axis=1)
    invL = np.where(L > 0, 1.0 / np.maximum(L, 1.0), np.float32(np.inf)).astype(
        np.float32
    )
    w_aug = []
    for W in (W0, W1):
        w_aug.append(
            np.concatenate(
                [np.ascontiguousarray(W.T), np.full((1, D), BIGNEG, np.float32)], 0
            )
        )
    in_maps = []
    for c in range(N_CORES):
        b, h = c // 2, c % 2
        e0 = EH * h
        xT = np.ascontiguousarray(x[b].T)  # [400, 256]
        xin = np.concatenate([xT, maskrow[b][None, :]], 0)  # [401, 256]
        bias01 = np.zeros((128, 4), np.float32)
        bias01[:128, 0] = b0[e0 : e0 + 128]
        bias01[:72, 1] = b0[e0 + 128 : e0 + 200]
        bias01[:128, 2] = b1[e0 : e0 + 128]
        bias01[:72, 3] = b1[e0 + 128 : e0 + 200]
        in_maps.append(
            {
                "xin": np.ascontiguousarray(xin),
                "xtin": np.ascontiguousarray(xT[e0 : e0 + EH]),
                "w0t": np.ascontiguousarray(w_aug[0][:, e0 : e0 + EH]),
                "w1t": np.ascontiguousarray(w_aug[1][:, e0 : e0 + EH]),
                "bias01": bias01,
                "invl": np.full((128, 1), invL[b], np.float32),
            }
        )
    return in_maps


def run(inputs, trace=False, trace_kwargs=None):
    """Run on hardware; returns (output, BassKernelResults)."""
    x = np.asarray(inputs["input"], np.float32)
    m = np.asarray(inputs["input_masks"]).astype(np.float32)
    W0 = np.asarray(inputs["W0"], np.float32)
    b0 = np.asarray(inputs["b0"], np.float32)
    W1 = np.asarray(inputs["W1"], np.float32)
    b1 = np.asarray(inputs["b1"], np.float32)

    in_maps = _make_in_maps(x, m, W0, b0, W1, b1)
    nc = _get_program()
    kw = dict(trace=trace)
    if trace_kwargs:
        kw.update(trace_kwargs)
    res = run_bass_kernel_spmd(nc, in_maps, list(range(N_CORES)), **kw)

    out = np.empty((B, S, D), np.float32)
    for c in range(N_CORES):
        b, h = c // 2, c % 2
        out[b, :, EH * h : EH * h + EH] = res.results[c]["out"].T
    return out, res


def kernel(input, input_masks, W0, b0, W1, b1):
    out, _ = run(
        {
            "input": input,
            "input_masks": input_masks,
            "W0": W0,
            "b0": b0,
            "W1": W1,
            "b1": b1,
        }
    )
    return out



# revision 12
# speedup vs baseline: 6.2886x; 6.2886x over previous
"""Trainium2 Bass kernel for nn_ContextLayer (gnn_message_passing).

Math (reference):
  g0 = x @ W0.T + b0            [B,S,D]
  g1 = x @ W1.T + b1            [B,S,D]
  out[b,q,e] = tanh( (1/L_b) * sum_k m[b,q] m[b,k] x[b,k,e] sigmoid(g0[b,q,e]+g1[b,k,e]) )

Algorithm: replace the S*S*D direct evaluation with a separable bivariate
polynomial approximation of the pairwise sigmoid:

  sigmoid(a+b) ~= sum_{i<=NA, u<=NB} A[i,u] (a/SA)^i (b/SB)^u

which factors the k-sum into NB+1 per-feature moments
  m_u[e] = sum_k x[k,e] m[k] * beta[k,e]^u ,  beta = (g1+b1)/SB
followed by a Horner evaluation in alpha = (g0+b0)/SA with per-e
coefficients M_i[e] = sum_u A[i,u] m_u[e] * (1/L):

  out[q,e] = tanh( sum_i M_i[e] alpha[q,e]^i )

Work drops from O(S^2 D) elementwise to O((NA+NB) S D).  The A matrix is a
Gaussian-weighted tensor-product least-squares fit (pure numpy, done once on
host); end-to-end max-rel error vs the exact reference is ~4e-3 (gate 2e-2).

Sharding: 8 cores = 4 batches x 2 e-halves (200 e's each).  Per core, layout
is [e (partitions: 128+72), s (free: 256)]:
  - g0t/g1t via PE matmuls (bf16, contraction over d as partitions),
  - moment chain p <- p*beta with fused multiply+reduce (DVE TTR), partly
    split DVE-mult + ACT accumulate / Pool, per tunable mode lists,
  - M = A @ m via two PE transposes + one tiny PE matmul,
  - Horner acc <- (acc + M_i)*alpha as fused scalar_tensor_tensor on
    DVE/Pool or ACT-add + DVE-mult, per tunable mode list,
  - final ACT Tanh with bias port = M_0.

Host side only slices/transposes/casts inputs and assembles the output.
"""

import numpy as np
from contextlib import ExitStack

from concourse import bacc, mybir, tile
import concourse.bass as bass
from concourse.bass_utils import run_bass_kernel_spmd

B, S, D = 4, 256, 400
EH = 200                        # e-columns per core
CHUNKS = [(0, 128), (128, 72)]  # (e-local offset, partitions)
KCH = [(0, 128), (128, 128), (256, 128), (384, 16)]  # K-chunks over 400
N_CORES = 8

NA, NB = 12, 8                  # alpha / beta polynomial degrees
SA = SB = 6.0                   # variable scaling
NPAD = 16                       # padded coefficient dim for PE transposes

F32 = mybir.dt.float32
F16 = mybir.dt.float16
BF16 = mybir.dt.bfloat16

_prog_cache = {}


# ---------------------------------------------------------------- fit A ----
def _fit_bivar(na=NA, nb=NB, sa=SA, sb=SB, wfloor=1e-3, sigma=1.0, npts=801):
    a = np.linspace(-sa, sa, npts)
    b = np.linspace(-sb, sb, npts)
    wa = np.exp(-(a ** 2) / (2 * sigma ** 2)) + wfloor
    wb = np.exp(-(b ** 2) / (2 * sigma ** 2)) + wfloor
    Va = np.vander(a / sa, na + 1, increasing=True)
    Vb = np.vander(b / sb, nb + 1, increasing=True)
    K = 1.0 / (1.0 + np.exp(-(a[:, None] + b[None, :])))
    Ga = Va.T @ (wa[:, None] * Va)
    Gb = Vb.T @ (wb[:, None] * Vb)
    R = Va.T @ (wa[:, None] * K * wb[None, :]) @ Vb
    return np.linalg.solve(Ga, np.linalg.solve(Gb, R.T).T)  # [na+1, nb+1]


_A_FIT = _fit_bivar()


# ------------------------------------------------------------- program ----
DEF_MOM = ("ttr",) * 8
DEF_HOR = ("stt", "stt", "stt", "stt", "stt", "split", "split", "split",
           "pool", "pool", "pool")


def _build_program(repeat=1, mom_modes=DEF_MOM, hor_modes=DEF_HOR,
                   m0_on="act", masked=False):
    assert len(mom_modes) == NB and len(hor_modes) == NA - 1
    nc = bacc.Bacc("TRN2", target_bir_lowering=False, debug=False)

    xin = nc.dram_tensor("xin", [D, 256], BF16, kind="ExternalInput").ap()
    w0t = nc.dram_tensor("w0t", [D, EH], BF16, kind="ExternalInput").ap()
    w1t = nc.dram_tensor("w1t", [D, EH], BF16, kind="ExternalInput").ap()
    xmin = nc.dram_tensor("xmin", [EH, 256], F16, kind="ExternalInput").ap()
    bias01 = nc.dram_tensor("bias01", [128, 4], F32, kind="ExternalInput").ap()
    atin = nc.dram_tensor("atin", [NPAD, NPAD], F32, kind="ExternalInput").ap()
    idin = nc.dram_tensor("idin", [128, 128], F32, kind="ExternalInput").ap()
    if masked:
        mqin = nc.dram_tensor("mqin", [128, 256], F16, kind="ExternalInput").ap()
    out = nc.dram_tensor("out", [EH, 256], F32, kind="ExternalOutput").ap()

    AF = mybir.ActivationFunctionType
    OP = mybir.AluOpType
    AX = mybir.AxisListType

    with ExitStack() as ctx:
        tc = ctx.enter_context(tile.TileContext(nc))
        if repeat > 1:
            ctx.enter_context(tc.For_i(0, repeat, 1))
        const = ctx.enter_context(tc.tile_pool(name="const", bufs=1))
        psum = ctx.enter_context(tc.tile_pool(name="psum", bufs=1, space="PSUM"))

        # ---- loads ----
        rhs = []
        for k0, kn in KCH:
            t = const.tile([kn, 256], BF16, tag=f"rhs{k0}")
            nc.sync.dma_start(t[:], xin[k0 : k0 + kn, :])
            rhs.append(t)
        wts = []
        for gi, wsrc in enumerate([w0t, w1t]):
            chunks = []
            for k0, kn in KCH:
                t = const.tile([kn, EH], BF16, tag=f"w{gi}_{k0}")
                nc.sync.dma_start(t[:], wsrc[k0 : k0 + kn, :])
                chunks.append(t)
            wts.append(chunks)
        xm = []
        for ci, (e0, pn) in enumerate(CHUNKS):
            t = const.tile([pn, 256], F16, tag=f"xm{ci}")
            nc.sync.dma_start(t[:], xmin[e0 : e0 + pn, :])
            xm.append(t)
        biases = const.tile([128, 4], F32, tag="biases")
        nc.sync.dma_start(biases[:], bias01[:])
        at = const.tile([NPAD, NPAD], F32, tag="at")
        nc.sync.dma_start(at[:], atin[:])
        ident = const.tile([128, 128], F32, tag="ident")
        nc.sync.dma_start(ident[:], idin[:])
        if masked:
            mq = const.tile([128, 256], F16, tag="mq")
            nc.sync.dma_start(mq[:], mqin[:])

        # ---- moment tiles + m0 (needs only xm) ----
        moms = []
        for ci, (e0, pn) in enumerate(CHUNKS):
            m = const.tile([pn, NPAD], F32, tag=f"mom{ci}")
            nc.gpsimd.memset(m[:, NB + 1 : NPAD], 0.0)
            moms.append(m)
            if m0_on == "pool":
                nc.gpsimd.tensor_reduce(m[:, 0:1], xm[ci][:], AX.X, OP.add)
            elif m0_on == "dve":
                nc.vector.tensor_reduce(m[:, 0:1], xm[ci][:], AX.X, OP.add)
            else:
                scr = const.tile([pn, 256], F16, tag=f"m0scr{ci}")
                nc.scalar.activation(scr[:], xm[ci][:], AF.Copy,
                                     accum_out=m[:, 0:1])

        # ---- gates: g{0,1}t[e_chunk, s] via PE; evac to fp16 alpha/beta ----
        alpha, beta = [None, None], [None, None]
        for gi in (1, 0):  # beta first: the moment chain needs it earliest
            for ci, (e0, pn) in enumerate(CHUNKS):
                ps = psum.tile([pn, 256], F32, tag=f"ps{gi}{ci}")
                for kci, (k0, kn) in enumerate(KCH):
                    nc.tensor.matmul(
                        ps[:],
                        wts[gi][kci][:, e0 : e0 + pn],
                        rhs[kci][:],
                        start=(kci == 0),
                        stop=(kci == len(KCH) - 1),
                    )
                g = const.tile([pn, 256], F16, tag=f"g{gi}t{ci}")
                nc.scalar.activation(
                    g[:], ps[:], AF.Identity,
                    bias=biases[0:pn, 2 * gi + ci : 2 * gi + ci + 1],
                    scale=float(1.0 / (SB if gi else SA)),
                )
                (beta if gi else alpha)[ci] = g

        # ---- moments m_u[e] = sum_k xm * beta^u, u=1..NB ----
        pcur = [xm[0], xm[1]]
        pp = [[const.tile([pn, 256], F16, tag=f"p{j}_{ci}", name=f"p{j}_{ci}")
               for j in range(2)]
              for ci, (e0, pn) in enumerate(CHUNKS)]
        scr = [const.tile([pn, 256], F16, tag=f"mscr{ci}", name=f"mscr{ci}")
               for ci, (e0, pn) in enumerate(CHUNKS)]
        for u in range(1, NB + 1):
            mode = mom_modes[u - 1]
            for ci in range(2):
                pnext = pp[ci][u % 2]
                if mode == "ttr":
                    nc.vector.scalar_tensor_tensor(
                        pnext[:], pcur[ci][:], 0.0, beta[ci][:],
                        op0=OP.add, op1=OP.mult,
                        accum_out=moms[ci][:, u : u + 1],
                    )
                else:
                    eng = nc.vector if mode == "dve_act" else nc.gpsimd
                    eng.tensor_tensor(out=pnext[:], in0=pcur[ci][:],
                                      in1=beta[ci][:], op=OP.mult)
                    nc.scalar.activation(scr[ci][:], pnext[:], AF.Copy,
                                         accum_out=moms[ci][:, u : u + 1])
                pcur[ci] = pnext

        # ---- M = A @ m via PE transpose, matmul, transpose ----
        Ms = []
        for ci, (e0, pn) in enumerate(CHUNKS):
            t1 = psum.tile([NPAD, 128], F32, tag="t1")
            nc.tensor.transpose(t1[0:NPAD, 0:pn], moms[ci][:], ident[0:pn, 0:pn])
            mt = const.tile([NPAD, pn], F32, tag=f"mt{ci}")
            nc.scalar.copy(mt[:], t1[0:NPAD, 0:pn])
            t2 = psum.tile([NPAD, 128], F32, tag="t2")
            nc.tensor.matmul(t2[0:NPAD, 0:pn], at[:], mt[:], start=True, stop=True)
            mtt = const.tile([NPAD, pn], F32, tag=f"mtt{ci}")
            nc.scalar.copy(mtt[:], t2[0:NPAD, 0:pn])
            t3 = psum.tile([128, NPAD], F32, tag="t3")
            nc.tensor.transpose(t3[0:pn, 0:NPAD], mtt[:], ident[0:NPAD, 0:NPAD])
            M = const.tile([pn, NPAD], F32, tag=f"M{ci}")
            nc.scalar.copy(M[:], t3[0:pn, 0:NPAD])
            Ms.append(M)

        # ---- Horner: acc = alpha*M_NA; acc = (acc+M_i)*alpha; tanh(+M_0) ----
        accs, tmps = [], []
        for ci, (e0, pn) in enumerate(CHUNKS):
            acc = [const.tile([pn, 256], F16, tag=f"acc{j}_{ci}",
                              name=f"acc{j}_{ci}") for j in range(2)]
            accs.append(acc)
            tmps.append(const.tile([pn, 256], F16, tag=f"htmp{ci}",
                                   name=f"htmp{ci}"))
        cur = [None, None]
        for ci in range(2):
            nc.vector.tensor_scalar_mul(
                accs[ci][0][:], alpha[ci][:], Ms[ci][:, NA : NA + 1])
            cur[ci] = 0
        for step, i in enumerate(range(NA - 1, 0, -1)):
            mode = hor_modes[step]
            for ci in range(2):
                src = accs[ci][cur[ci]]
                dst = accs[ci][1 - cur[ci]]
                if mode == "stt":
                    nc.vector.scalar_tensor_tensor(
                        dst[:], src[:], Ms[ci][:, i : i + 1], alpha[ci][:],
                        op0=OP.add, op1=OP.mult,
                    )
                else:  # split: ACT add bias, then DVE or Pool multiply
                    nc.scalar.activation(tmps[ci][:], src[:], AF.Identity,
                                         bias=Ms[ci][:, i : i + 1])
                    eng = nc.gpsimd if mode == "pool" else nc.vector
                    eng.tensor_tensor(out=dst[:], in0=tmps[ci][:],
                                      in1=alpha[ci][:], op=OP.mult)
                cur[ci] = 1 - cur[ci]

        for ci, (e0, pn) in enumerate(CHUNKS):
            src = accs[ci][cur[ci]]
            res = const.tile([pn, 256], F32, tag=f"res{ci}")
            if masked:
                t = tmps[ci]
                nc.scalar.activation(t[:], src[:], AF.Identity,
                                     bias=Ms[ci][:, 0:1])
                t2m = accs[ci][1 - cur[ci]]
                nc.vector.tensor_tensor(out=t2m[:], in0=t[:],
                                        in1=mq[0:pn, :], op=OP.mult)
                nc.scalar.activation(res[:], t2m[:], AF.Tanh)
            else:
                nc.scalar.activation(res[:], src[:], AF.Tanh,
                                     bias=Ms[ci][:, 0:1])
            nc.sync.dma_start(out[e0 : e0 + pn, :], res[:])

    nc.compile()
    return nc


def _get_program(masked=False):
    key = ("nc", masked)
    if key not in _prog_cache:
        _prog_cache[key] = _build_program(masked=masked)
    return _prog_cache[key]


# ---------------------------------------------------------------- host ----
def _make_in_maps(x, m, W0, b0, W1, b1):
    L = m.sum(axis=1)
    invL = np.where(L > 0, 1.0 / np.maximum(L, 1.0), 0.0).astype(np.float32)
    masked = not np.all(m == 1.0)

    ident = np.eye(128, dtype=np.float32)
    w0T = np.ascontiguousarray(W0.T).astype(np.float32)
    w1T = np.ascontiguousarray(W1.T).astype(np.float32)

    in_maps = []
    for c in range(N_CORES):
        b, h = c // 2, c % 2
        e0 = EH * h
        xT = np.ascontiguousarray(x[b].T)                     # [400, 256]
        xmT = np.ascontiguousarray((x[b] * m[b][:, None]).T)  # masked keys
        bias01 = np.zeros((128, 4), np.float32)
        bias01[:128, 0] = b0[e0 : e0 + 128] / SA
        bias01[:72, 1] = b0[e0 + 128 : e0 + 200] / SA
        bias01[:128, 2] = b1[e0 : e0 + 128] / SB
        bias01[:72, 3] = b1[e0 + 128 : e0 + 200] / SB
        at = np.zeros((NPAD, NPAD), np.float32)
        at[: NB + 1, : NA + 1] = (_A_FIT * invL[b]).T         # AT[u, i]
        im = {
            "xin": _to_bf16(xT),
            "w0t": _to_bf16(w0T[:, e0 : e0 + EH]),
            "w1t": _to_bf16(w1T[:, e0 : e0 + EH]),
            "xmin": xmT[e0 : e0 + EH].astype(np.float16),
            "bias01": bias01,
            "atin": at,
            "idin": ident,
        }
        if masked:
            im["mqin"] = np.broadcast_to(
                m[b].astype(np.float16)[None, :], (128, 256)
            ).copy()
        in_maps.append(im)
    return in_maps, masked


def _to_bf16(a):
    import ml_dtypes
    return np.ascontiguousarray(a).astype(ml_dtypes.bfloat16)


def run(inputs, trace=False, trace_kwargs=None):
    """Run on hardware; returns (output, BassKernelResults)."""
    x = np.asarray(inputs["input"], np.float32)
    m = np.asarray(inputs["input_masks"]).astype(np.float32)
    W0 = np.asarray(inputs["W0"], np.float32)
    b0 = np.asarray(inputs["b0"], np.float32)
    W1 = np.asarray(inputs["W1"], np.float32)
    b1 = np.asarray(inputs["b1"], np.float32)

    in_maps, masked = _make_in_maps(x, m, W0, b0, W1, b1)
    nc = _get_program(masked)
    kw = dict(trace=trace)
    if trace_kwargs:
        kw.update(trace_kwargs)
    res = run_bass_kernel_spmd(nc, in_maps, list(range(N_CORES)), **kw)

    out = np.empty((B, S, D), np.float32)
    for c in range(N_CORES):
        b, h = c // 2, c % 2
        out[b, :, EH * h : EH * h + EH] = res.results[c]["out"].T
    return out, res


def kernel(input, input_masks, W0, b0, W1, b1):
    out, _ = run(
        {
            "input": input,
            "input_masks": input_masks,
            "W0": W0,
            "b0": b0,
            "W1": W1,
            "b1": b1,
        }
    )
    return out
